# revision 2
# baseline (speedup 1.0000x reference)
"""Swin-style windowed MHA kernel for 8 Trainium2 NeuronCores — v3.

Same math/layout as v2 but the emission loop is software-pipelined in three
stages so each engine's in-order SEQ stream rarely stalls (head-of-line
blocking was the v2 bottleneck):
  A(S):   DMA-in, LN (bn_stats + Newton rsqrt + apply), transpose,
          q/k/v/v2 matmuls + PSUM evacuations
  B(S-1): block-diag K^T build, bias preload + sims, exp, attnV(+denoms)
  C(S-2): normalize (divide), O transposes, out-proj, DMA-out
Iteration `it` emits A(it), B(it-1), C(it-2).
"""

import numpy as np
import ml_dtypes

try:
    import concourse.bass as _b  # noqa: F401
except Exception:  # pragma: no cover
    import sys
    sys.path.insert(0, "/opt/trn_rl_repo")

BF = ml_dtypes.bfloat16

B_, X_, Y_, W1, W2, D = 16, 16, 16, 8, 8, 128
NWIN = B_ * X_ * Y_
NTOK = W1 * W2
HEADS = 4
DH = D // HEADS
NC_CORES = 8
WPC = NWIN // NC_CORES
SBW = 8
NSB = WPC // SBW             # 64
SBT = SBW * NTOK             # 512
LN_EPS = 1e-5


def _host_prep(gamma, beta, w_qkv, w_out, bias_table):
    scale = DH ** -0.5
    C = np.eye(D) - np.full((D, D), 1.0 / D)
    W = C @ (np.diag(gamma.astype(np.float64)) @ w_qkv.astype(np.float64))
    Wq = (W[:, :D] * scale).astype(BF)
    Wk = W[:, D:2 * D].astype(BF)
    Wv = W[:, 2 * D:].astype(BF)
    qb = beta.astype(np.float64) @ w_qkv.astype(np.float64)
    qb3 = np.stack([qb[:D] * scale, qb[D:2 * D], qb[2 * D:]], axis=1).astype(np.float32)

    # OT partition order: 64*bh + 2*c + hp  <-  channel (2hp+bh)*32 + c
    perm = np.empty(D, np.int64)
    for bh in range(2):
        for c in range(DH):
            for hp in range(2):
                perm[64 * bh + 2 * c + hp] = (2 * hp + bh) * DH + c
    WoP = w_out.astype(np.float64)[perm, :].astype(BF)

    # Badd [128, 1024]: [(bh, j) parts, (hp 2, w 8, i 64) cols], B_h[i, j]
    gh = np.arange(W1)
    gw = np.arange(W2)
    grid = np.stack(np.meshgrid(gh, gw, indexing="ij"), -1).reshape(-1, 2)
    rel = grid[:, None, :] - grid[None, :, :] + np.array([W1 - 1, W2 - 1])
    idx = rel[..., 0] * (2 * W2 - 1) + rel[..., 1]
    Bh = bias_table[idx].astype(np.float64)
    Badd = np.zeros((128, 1024), np.float64)
    for bh in range(2):
        for hp in range(2):
            h = 2 * hp + bh
            for w in range(8):
                Badd[64 * bh:64 * bh + 64,
                     512 * hp + 64 * w:512 * hp + 64 * w + 64] = Bh[:, :, h].T
    Badd = Badd.astype(BF)
    return Wq, Wk, Wv, WoP, Badd, qb3


def _build(has_qb=False):
    import concourse.bacc as bacc
    import concourse.mybir as mybir
    import concourse.tile as tile
    from concourse.masks import make_identity
    from contextlib import ExitStack

    f32 = mybir.dt.float32
    bf16 = mybir.dt.bfloat16
    AF = mybir.ActivationFunctionType
    ALU = mybir.AluOpType

    nc = bacc.Bacc()
    x_d = nc.declare_dram_parameter("x", (NSB * 128, SBT), bf16, isOutput=False)
    wq_d = nc.declare_dram_parameter("wq", (D, D), bf16, isOutput=False)
    wk_d = nc.declare_dram_parameter("wk", (D, D), bf16, isOutput=False)
    wv_d = nc.declare_dram_parameter("wv", (D, D), bf16, isOutput=False)
    wo_d = nc.declare_dram_parameter("wo", (D, D), bf16, isOutput=False)
    eb_d = nc.declare_dram_parameter("Badd", (128, 1024), bf16, isOutput=False)
    out_d = nc.declare_dram_parameter("out", (NSB * 128, SBT), bf16, isOutput=True)

    with tile.TileContext(nc) as tc, ExitStack() as ctx:
        cp = ctx.enter_context(tc.tile_pool(name="const", bufs=1))
        sp = ctx.enter_context(tc.tile_pool(name="sb", bufs=4))
        pp = ctx.enter_context(tc.tile_pool(name="ps", bufs=1, space="PSUM"))

        wq_s = cp.tile([128, 128], bf16)
        wk_s = cp.tile([128, 128], bf16)
        wv_s = cp.tile([128, 128], bf16)
        wo_s = cp.tile([128, 128], bf16)
        eb_s = cp.tile([128, 1024], bf16)
        nc.sync.dma_start(wq_s[:], wq_d[:, :])
        nc.sync.dma_start(wk_s[:], wk_d[:, :])
        nc.sync.dma_start(wv_s[:], wv_d[:, :])
        nc.sync.dma_start(wo_s[:], wo_d[:, :])
        nc.sync.dma_start(eb_s[:], eb_d[:, :])
        ident = cp.tile([128, 128], bf16)
        make_identity(nc, ident[:])

        vs_t = [cp.tile([128, 1056], bf16, name=f"vs{i}", tag=f"vs{i}")
                for i in range(3)]
        for t in vs_t:
            nc.vector.memset(t[:], 0.0)
            for half in range(2):
                ones_ap = t[:, 528 * half:528 * (half + 1)].rearrange(
                    "p (r q c) -> p r q c", r=4, q=4)[:, :, :, 32]
                nc.vector.memset(ones_ap, 1.0)
        kbd_t = [cp.tile([128, 1024], bf16, name=f"kbd{i}", tag=f"kbd{i}")
                 for i in range(2)]
        for t in kbd_t:
            nc.vector.memset(t[:], 0.0)

        state = {}

        def stage_a(S):
            vs = vs_t[S % 3]
            xs = sp.tile([128, SBT], bf16, tag="xs")
            nc.sync.dma_start(xs[:], x_d[128 * S:128 * (S + 1), :])

            st6 = sp.tile([128, 24], f32, tag="st6")
            mv = sp.tile([128, 8], f32, tag="mv")
            sd4 = sp.tile([128, 4], f32, tag="sd4")
            rr4 = sp.tile([128, 4], f32, tag="rr4")
            var4 = sp.tile([128, 4], f32, tag="var4")
            for g in range(4):
                nc.vector.bn_stats(st6[:, 6 * g:6 * g + 6],
                                   xs[:, 128 * g:128 * (g + 1)])
            for g in range(4):
                nc.vector.bn_aggr(mv[:, 2 * g:2 * g + 2], st6[:, 6 * g:6 * g + 6])
            # Newton rsqrt, all on Pool (same-engine chain, Pool is idle)
            nc.gpsimd.tensor_scalar(
                var4[:], mv[:].rearrange("p (g s) -> p g s", s=2)[:, :, 1],
                1.0, LN_EPS, op0=ALU.mult, op1=ALU.add)
            nc.gpsimd.tensor_scalar(rr4[:], var4[:], -0.5, 1.5,
                                    op0=ALU.mult, op1=ALU.add)
            for _it in range(2):
                nc.gpsimd.tensor_mul(sd4[:], rr4[:], rr4[:])
                nc.gpsimd.tensor_mul(sd4[:], sd4[:], var4[:])
                nc.gpsimd.tensor_scalar(sd4[:], sd4[:], -0.5, 1.5,
                                        op0=ALU.mult, op1=ALU.add)
                nc.gpsimd.tensor_mul(rr4[:], rr4[:], sd4[:])

            xn = sp.tile([128, SBT], bf16, tag="xn")
            for g in range(4):
                o = xn[:, 128 * g:128 * (g + 1)]
                i = xs[:, 128 * g:128 * (g + 1)]
                sc = rr4[:, g:g + 1]
                nc.gpsimd.tensor_scalar(o, i, sc, None, op0=ALU.mult)

            xnT_ps = pp.tile([128, SBT], bf16, tag="pb", bufs=2)
            for g in range(4):
                nc.tensor.transpose(xnT_ps[:, 128 * g:128 * (g + 1)],
                                    xn[:, 128 * g:128 * (g + 1)], ident[:])
            xnT_s = sp.tile([128, SBT], bf16, tag="xnT")
            nc.vector.tensor_copy(xnT_s[:], xnT_ps[:])

            q_ps = pp.tile([128, SBT], f32, tag="pf", bufs=3)
            k_ps = pp.tile([128, SBT], f32, tag="pf", bufs=3)
            nc.tensor.matmul(q_ps[:], wq_s[:], xnT_s[:], start=True, stop=True)
            nc.tensor.matmul(k_ps[:], wk_s[:], xnT_s[:], start=True, stop=True)
            qkT_s = sp.tile([128, 1024], bf16, tag="qkT")
            nc.scalar.activation(qkT_s[:, 0:512], q_ps[:], AF.Copy)
            nc.scalar.activation(qkT_s[:, 512:1024], k_ps[:], AF.Copy)

            v_ps = pp.tile([128, SBT], f32, tag="pf", bufs=3)
            for g in range(4):
                nc.tensor.matmul(v_ps[:, 128 * g:128 * (g + 1)],
                                 xnT_s[:, 128 * g:128 * (g + 1)], wv_s[:],
                                 start=True, stop=True)
            vin = v_ps[:].rearrange("p (r h c) -> p r h c", r=4, h=4)
            vout = vs[:, 0:528].rearrange(
                "p (r q c) -> p r q c", r=4, q=4)[:, :, :, 0:32]
            nc.scalar.activation(vout, vin, AF.Copy)

            # partition-swapped replica for the attnV bh!=wp slices; the
            # pipeline gives this a full iteration of slack to complete
            nc.sync.dma_start(vs[64:128, 528:1056], vs[0:64, 0:528])
            nc.sync.dma_start(vs[0:64, 528:1056], vs[64:128, 0:528])
            state[S] = dict(qkT_s=qkT_s)

        def stage_b(S):
            vs = vs_t[S % 3]
            kbd = kbd_t[S % 2]
            qkT_s = state[S]["qkT_s"]
            for hp in range(2):
                for half in range(2):
                    r0 = 64 * hp + 32 * half
                    src = qkT_s[r0:r0 + 32, 512:1024] \
                        .rearrange("p (w j) -> p w j", j=64)
                    dst = kbd[r0:r0 + 32, :].rearrange(
                        "p (w j) -> p w j", j=128)[:, :, 64 * half:64 * half + 64]
                    if hp == 1 and half == 1:
                        nc.gpsimd.tensor_copy(dst, src)
                    else:
                        nc.vector.tensor_copy(dst, src)
            sim_ps = [pp.tile([128, 512], f32, name=f"sim{_t}", tag="pf", bufs=3)
                      for _t in range(2)]
            for hp in range(2):
                nc.tensor.matmul(sim_ps[hp][:], ident[:],
                                 eb_s[:, 512 * hp:512 * hp + 512],
                                 start=True, stop=False, skip_group_check=True)
            for w in range(8):
                for hp in range(2):
                    nc.tensor.matmul(
                        sim_ps[hp][:, 64 * w:64 * w + 64],
                        kbd[64 * hp:64 * hp + 64, 128 * w:128 * w + 128],
                        qkT_s[64 * hp:64 * hp + 64, 64 * w:64 * w + 64],
                        start=False, stop=True,
                        tile_position=(64 * hp, 0), skip_group_check=True)
            PT = sp.tile([128, 1024], bf16, tag="PT")
            for hp in range(2):
                nc.scalar.activation(PT[:, 512 * hp:512 * (hp + 1)],
                                     sim_ps[hp][:], AF.Exp)

            O_ps = [pp.tile([128, 264], f32, name=f"O{_t}", tag="pO", bufs=3)
                    for _t in range(2)]
            for w in range(8):
                wp, pr = w % 2, w // 2
                for h in range(4):
                    hp, bh = h // 2, h % 2
                    cb = 512 * hp + 64 * w
                    voff = 0 if bh == wp else 528
                    outap = O_ps[bh][64 * wp:64 * wp + 64, 66 * pr:66 * pr + 66]
                    outap = outap.rearrange("p (c f) -> p c f", f=2)[:, :, hp]
                    nc.tensor.matmul(
                        outap,
                        PT[64 * bh:64 * bh + 64, cb:cb + 64],
                        vs[64 * bh:64 * bh + 64,
                           voff + 132 * pr + 33 * h:voff + 132 * pr + 33 * h + 33],
                        start=True, stop=True,
                        tile_position=(64 * bh, 64 * wp), skip_group_check=True)
            state[S]["O_ps"] = O_ps

        def stage_c(S):
            O_ps = state[S]["O_ps"]
            O_sb = sp.tile([128, SBT], bf16, tag="Osb")
            sr = sp.tile([128, 16], f32, tag="sr")
            for bh in range(2):
                sin = O_ps[bh][:].rearrange("p (r c) -> p r c", r=4)[:, :, 64:66]
                nc.vector.reciprocal(
                    sr[:, 8 * bh:8 * bh + 8].rearrange("p (r f) -> p r f", r=4),
                    sin)
                in0 = O_ps[bh][:].rearrange(
                    "p (r c f) -> p r c f", r=4, f=2)[:, :, 0:32, :]
                in1 = sr[:, 8 * bh:8 * bh + 8] \
                    .rearrange("p (r f) -> p r f", r=4).unsqueeze(2) \
                    .to_broadcast((128, 4, 32, 2))
                out = O_sb[:].rearrange(
                    "p (r b c f) -> p r b c f", r=4, b=2, f=2)[:, :, bh, :, :]
                nc.vector.tensor_tensor(out=out, in0=in0, in1=in1, op=ALU.mult)

            OT_ps = pp.tile([128, SBT], bf16, tag="pb", bufs=2)
            for pr in range(4):
                nc.tensor.transpose(OT_ps[:, 128 * pr:128 * (pr + 1)],
                                    O_sb[:, 128 * pr:128 * (pr + 1)], ident[:])
            OT_s = sp.tile([128, SBT], bf16, tag="OT")
            nc.vector.tensor_copy(OT_s[:], OT_ps[:])

            y_ps = pp.tile([128, SBT], f32, tag="pf", bufs=3)
            nc.tensor.matmul(y_ps[:], wo_s[:], OT_s[:], start=True, stop=True)
            y_sb = sp.tile([128, SBT], bf16, tag="ysb")
            nc.scalar.activation(y_sb[:], y_ps[:], AF.Copy)
            nc.sync.dma_start(out_d[128 * S:128 * (S + 1), :], y_sb[:])
            del state[S]

        for it in range(NSB + 2):
            if it < NSB:
                stage_a(it)
            if 1 <= it <= NSB:
                stage_b(it - 1)
            if 2 <= it:
                stage_c(it - 2)

    nc.compile()
    return nc


def _pack_x(x_shard):
    xr = x_shard.reshape(NSB, 4, 2, 64, D)
    xr = xr.transpose(0, 2, 3, 1, 4)
    return np.ascontiguousarray(xr.reshape(NSB * 128, 4 * D).astype(BF))


def _unpack_y(y_core):
    y = y_core.reshape(NSB, 128, SBT).astype(np.float32)
    y = y.transpose(0, 2, 1)
    return y.reshape(WPC, NTOK, D)


def kernel(**inputs):
    x = np.asarray(inputs["x"], np.float32)
    gamma = np.asarray(inputs["gamma"], np.float32)
    beta = np.asarray(inputs["beta"], np.float32)
    w_qkv = np.asarray(inputs["w_qkv"], np.float32)
    w_out = np.asarray(inputs["w_out"], np.float32)
    bias_table = np.asarray(inputs["bias_table"], np.float32)

    from concourse.bass_utils import run_bass_kernel_spmd

    Wq, Wk, Wv, WoP, Badd, qb3 = _host_prep(gamma, beta, w_qkv, w_out, bias_table)
    assert not np.any(qb3), "beta != 0 path not implemented"

    nc = _build(False)

    xr = x.reshape(NWIN, NTOK, D)
    in_maps = []
    for c in range(NC_CORES):
        shard = _pack_x(xr[WPC * c:WPC * (c + 1)])
        in_maps.append(dict(x=shard, wq=Wq, wk=Wk, wv=Wv, wo=WoP, Badd=Badd))

    res = run_bass_kernel_spmd(nc, in_maps, core_ids=list(range(NC_CORES)))
    outs = [_unpack_y(res.results[c]["out"]) for c in range(NC_CORES)]
    y = np.concatenate(outs, axis=0).reshape(B_, X_, Y_, W1, W2, D)
    return np.ascontiguousarray(y, dtype=np.float32)
